# revision 16
# baseline (speedup 1.0000x reference)
"""Trainium2 Bass kernel for nn_AdderConv (8-layer AdderNet CNN).

Problem: 8 adder-conv blocks (|x-w| sum pseudo-conv, 3x3, pad 1) + eval-mode
BN + ReLU, MaxPool2d(2,2) after blocks 1,3,7. Input x [64,3,32,32] f32,
output [64,128,4,4] f32.

Strategy (pure data parallelism, 8 images per NeuronCore):
  - Layout: x stored in SBUF as [(ci*nsub+isub) partitions, (g, Hp, Wp)] bf16
    with zero-padded borders; nsub images packed on partitions so that
    K = Cin*nsub <= 128 and M = Cout*nsub == 128 always.
  - Per (output-channel co, 3x3 tap, q-tile): VectorE tensor_scalar computes
    scratch[k, q] = |x_view - w| in one pass (op0=add with -w, op1=abs_max 0),
    bf16 -> runs in the DVE 4x perf mode.
  - TensorE reduces scratch over partitions via a one-hot selector matmul,
    accumulating all (co, tap) into one PSUM bank [128, 512]; rows for other
    co just accumulate zeros.
  - ScalarE evicts PSUM with fused BN+ReLU (activation Relu, per-partition
    scale=-gamma/sqrt(var+eps), bias=beta-mean*inv), writing bf16 into the
    next layer's padded buffer. MaxPool runs on PSUM before BN (valid since
    inv>0) via two strided tensor_tensor max steps.
  - A +1-element-shifted duplicate of each x buffer keeps the kw=1 taps
    4-byte aligned so the DVE stays in its fast mode.

All weight/BN/selector tables are prepared host-side in numpy (cheap, O(weights))
and shipped as extra DRAM inputs; all x-dependent compute runs on device.
"""

import os
import numpy as np

# ---------------- hardcoded problem config ----------------
N_CORES = 8
N_LOC = 8  # images per core
CONFIGS = [(3, 32), (32, 32), (32, 64), (64, 64), (64, 128), (128, 128), (128, 128), (128, 128)]
POOL_AFTER = (1, 3, 7)
BN_EPS = 1e-5

# per-layer static plan
# nsub: images packed on the partition dim;  G = N_LOC // nsub image groups in
# the free dim.  K = Cin*nsub (<=128), M = Cout*nsub (==128 for every layer).
_LAYERS = []
for _i, (_cin, _cout) in enumerate(CONFIGS):
    _H = 32 if _i < 2 else (16 if _i < 4 else 8)
    _nsub = 4 if _i < 2 else (2 if _i < 4 else 1)
    _LAYERS.append(dict(
        Cin=_cin, Cout=_cout, H=_H, W=_H, nsub=_nsub, G=N_LOC // _nsub,
        Hp=_H + 2, Wp=_H + 2, SZ=(_H + 2) * (_H + 2),
        K=_cin * _nsub, M=_cout * _nsub, pool=(_i in POOL_AFTER),
    ))
for _l in _LAYERS:
    assert _l["M"] == 128


def _qtiles(l):
    """q-tiles of 512 positions: list of (g0, Gt, h0, Ht)."""
    H, W, G = l["H"], l["W"], l["G"]
    if H == 32:      # G=2: (g, 16 rows) tiles
        return [(g, 1, h0, 16) for g in range(G) for h0 in (0, 16)]
    elif H == 16:    # G=4: (2 image groups, full 16x16)
        return [(g0, 2, 0, 16) for g0 in (0, 2)]
    else:            # H=8, G=8: everything in one tile
        return [(0, 8, 0, 8)]


# ---------------- host-side table prep ----------------

def _prep_tables(params):
    import ml_dtypes
    bf16 = ml_dtypes.bfloat16
    tabs = []
    for li, l in enumerate(_LAYERS):
        p = params[li]
        w = np.asarray(p["w"], np.float32)          # [Cout, Cin, 3, 3]
        gamma = np.asarray(p["gamma"], np.float32)
        beta = np.asarray(p["beta"], np.float32)
        mean = np.asarray(p["mean"], np.float32)
        var = np.asarray(p["var"], np.float32)
        inv = gamma / np.sqrt(var + BN_EPS)
        assert np.all(inv > 0), "pool-before-BN requires positive BN scale"
        Cin, Cout, nsub, K = l["Cin"], l["Cout"], l["nsub"], l["K"]
        # isub-MAJOR packing: k = isub*Cin + ci, psum row m = isub*Cout + co
        # wcol[k, co*9+tap] = -w[co, ci(k), kh, kw]
        ci_of_k = np.arange(K) % Cin
        wcol = -w[:, ci_of_k, :, :].transpose(1, 0, 2, 3).reshape(K, Cout * 9)
        # selector Z[k, f] = 1[f == F0 + isub(k)*Cout]; lhsT for co is
        # Z[:, F0-co : F0-co+128]
        F0 = Cout - 1
        # scratch holds relu(x-w); Sum|x-w| = 2*Sum relu(x-w) - Sum x + Sum w.
        # The 2x rides the selector; -Sum x comes from 9 extra matmuls with zn;
        # +Sum w folds into the BN bias.
        Z = np.zeros((K, F0 + 128), np.float32)
        Z[np.arange(K), F0 + (np.arange(K) // Cin) * Cout] = 2.0
        # zn[k, m] = -1 iff isub(m) == isub(k): subtracts the patch sum from
        # every co row of the matching image
        zn = np.zeros((K, 128), np.float32)
        zn[(np.arange(K) // Cin)[:, None] == (np.arange(128) // Cout)[None, :]] = -1.0
        # BN tables per psum row m = isub*Cout + co
        co_of_m = np.arange(128) % Cout
        W1 = w.reshape(Cout, -1).sum(axis=1)
        bns = (-inv[co_of_m]).reshape(128, 1).astype(np.float32)
        bnb = (beta - inv * (mean + W1))[co_of_m].reshape(128, 1).astype(np.float32)
        tabs.append(dict(
            wcol=np.ascontiguousarray(wcol.astype(np.float32)),
            Z=np.ascontiguousarray(Z.astype(bf16)),
            zn=np.ascontiguousarray(zn.astype(bf16)),
            bns=bns, bnb=bnb,
        ))
    return tabs


# ---------------- bass program ----------------

_NC_CACHE = {}
LAST_RESULTS = None  # BassKernelResults of the most recent run (for profiling)


def _build_nc():
    import concourse.bacc as bacc
    import concourse.bass as bass
    import concourse.tile as tile
    import concourse.mybir as mybir
    dt = mybir.dt
    Alu = mybir.AluOpType
    Act = mybir.ActivationFunctionType

    nc = bacc.Bacc(None)

    x_h = nc.declare_dram_parameter("x", [N_LOC, 3, 32, 32], dt.float32, isOutput=False)
    out_h = nc.declare_dram_parameter("out", [N_LOC, 128, 4, 4], dt.float32, isOutput=True)
    wcol_h, z_h, zn_h, bns_h, bnb_h = [], [], [], [], []
    for li, l in enumerate(_LAYERS):
        K, Cout, nsub = l["K"], l["Cout"], l["nsub"]
        F0 = Cout - 1
        wcol_h.append(nc.declare_dram_parameter(f"wcol{li}", [K, Cout * 9], dt.float32, isOutput=False))
        z_h.append(nc.declare_dram_parameter(f"z{li}", [K, F0 + 128], dt.bfloat16, isOutput=False))
        zn_h.append(nc.declare_dram_parameter(f"zn{li}", [K, 128], dt.bfloat16, isOutput=False))
        bns_h.append(nc.declare_dram_parameter(f"bns{li}", [128, 1], dt.float32, isOutput=False))
        bnb_h.append(nc.declare_dram_parameter(f"bnb{li}", [128, 1], dt.float32, isOutput=False))

    with tile.TileContext(nc) as tc:
        with (
            tc.tile_pool(name="persist", bufs=1) as persist,
            tc.tile_pool(name="scratch", bufs=8) as scratch_pool,
            tc.tile_pool(name="psum", bufs=2, space="PSUM") as psum_pool,
            tc.tile_pool(name="tmp0", bufs=2) as tmp0_pool,
            tc.tile_pool(name="tmp1", bufs=2) as tmp1_pool,
            tc.tile_pool(name="tmp2", bufs=2) as tmp2_pool,
            tc.tile_pool(name="tmpe", bufs=2) as tmpe_pool,
        ):
            # ---- persistent buffers ----
            x4 = []    # padded activations per layer
            dup = []   # +1-element shifted duplicates
            for li, l in enumerate(_LAYERS):
                shp = [l["K"], l["G"] * l["SZ"]]
                x4.append(persist.tile(shp, dt.bfloat16, name=f"x4_{li}", tag=f"x4_{li}"))
                dup.append(persist.tile(shp, dt.bfloat16, name=f"dup_{li}", tag=f"dup_{li}"))
            wcol_t, z_t, zn_t, bns_t, bnb_t = [], [], [], [], []
            for li, l in enumerate(_LAYERS):
                K, Cout, nsub = l["K"], l["Cout"], l["nsub"]
                F0 = Cout - 1
                wcol_t.append(persist.tile([K, Cout * 9], dt.float32, name=f"wc_{li}", tag=f"wc_{li}"))
                z_t.append(persist.tile([K, F0 + 128], dt.bfloat16, name=f"z_{li}", tag=f"z_{li}"))
                zn_t.append(persist.tile([K, 128], dt.bfloat16, name=f"zn_{li}", tag=f"zn_{li}"))
                bns_t.append(persist.tile([128, 1], dt.float32, name=f"bns_{li}", tag=f"bns_{li}"))
                bnb_t.append(persist.tile([128, 1], dt.float32, name=f"bnb_{li}", tag=f"bnb_{li}"))
            stage = persist.tile([12, 2 * 32 * 32], dt.float32, tag="stage")
            out_sb = persist.tile([128, N_LOC * 16], dt.float32, tag="out_sb")

            # ---- load tables ----
            for li in range(8):
                nc.gpsimd.dma_start(out=wcol_t[li][:], in_=wcol_h[li][:])
                nc.gpsimd.dma_start(out=z_t[li][:], in_=z_h[li][:])
                nc.gpsimd.dma_start(out=zn_t[li][:], in_=zn_h[li][:])
                nc.gpsimd.dma_start(out=bns_t[li][:], in_=bns_h[li][:])
                nc.gpsimd.dma_start(out=bnb_t[li][:], in_=bnb_h[li][:])

            # ---- zero-init buffers whose borders are never written ----
            for li in range(8):
                nc.gpsimd.memset(x4[li][:], 0.0)
            # dup borders must also be zero; initialise by copying the zeroed
            # x4 (single-wait DMAs; a memset would add a second wait on the
            # eviction-refresh DMAs, which DMA instructions cannot encode)
            for li in range(1, 8):
                nc.gpsimd.dma_start(out=dup[li][:], in_=x4[li][:])

            # ---- input: DMA x into dense fp32 stage, convert+pad to bf16 ----
            # stage layout [k = isub*3+ci, (g=2, 32, 32)];  image i = g*4+isub
            stage_r = stage[:].rearrange("p (g a b) -> p g a b", g=2, a=32, b=32)
            x_ap = x_h[:]
            for k in range(12):
                isub, ci = k // 3, k % 3
                for g in range(2):
                    nc.gpsimd.dma_start(
                        out=stage_r[k:k + 1, g, :, :],
                        in_=x_ap[g * 4 + isub, ci:ci + 1, :, :],
                    )
            x40_r = x4[0][:].rearrange("p (g a b) -> p g a b", g=2, a=34, b=34)
            nc.vector.tensor_copy(out=x40_r[0:12, :, 1:33, 1:33], in_=stage_r)
            nc.gpsimd.dma_start(out=dup[0][:, 0:2311], in_=x4[0][:, 1:2312])

            # ---- layers ----
            for li, l in enumerate(_LAYERS):
                K, Cout, nsub, G = l["K"], l["Cout"], l["nsub"], l["G"]
                H, W, Hp, Wp, SZ = l["H"], l["W"], l["Hp"], l["Wp"], l["SZ"]
                F0 = Cout - 1
                x4_r = x4[li][:].rearrange("p (g a b) -> p g a b", g=G, a=Hp, b=Wp)
                dup_r = dup[li][:].rearrange("p (g a b) -> p g a b", g=G, a=Hp, b=Wp)
                last = (li == 7)
                if not l["pool"]:
                    ln = _LAYERS[li + 1]
                    x4n_r = x4[li + 1][:].rearrange("p (g a b) -> p g a b", g=ln["G"], a=ln["Hp"], b=ln["Wp"])
                    dupn_r = dup[li + 1][:].rearrange("p (g a b) -> p g a b", g=ln["G"], a=ln["Hp"], b=ln["Wp"])
                elif not last:
                    ln = _LAYERS[li + 1]
                    x4n_r = x4[li + 1][:].rearrange("p (g a b) -> p g a b", g=ln["G"], a=ln["Hp"], b=ln["Wp"])
                    dupn_r = dup[li + 1][:].rearrange("p (g a b) -> p g a b", g=ln["G"], a=ln["Hp"], b=ln["Wp"])

                for (g0, Gt, h0, Ht) in _qtiles(l):
                    ps = psum_pool.tile([128, 512], dt.float32, name="ps", tag="ps")
                    n_mm = Cout * 9 + 9
                    mm = 0
                    # -Sum_j x_j(q): co-independent patch sum, negated via zn
                    for tap in range(9):
                        kh, kw = tap // 3, tap % 3
                        if kw == 1:
                            src = dup_r[0:K, g0:g0 + Gt, h0 + kh:h0 + kh + Ht, 0:W]
                        else:
                            src = x4_r[0:K, g0:g0 + Gt, h0 + kh:h0 + kh + Ht, kw:kw + W]
                        nc.tensor.matmul(
                            out=ps[:, :], lhsT=zn_t[li][0:K, 0:128],
                            rhs=src, start=(mm == 0), stop=False,
                        )
                        mm += 1
                    for co in range(Cout):
                        zoff = F0 - co
                        for tap in range(9):
                            kh, kw = tap // 3, tap % 3
                            if kw == 1:
                                src = dup_r[0:K, g0:g0 + Gt, h0 + kh:h0 + kh + Ht, 0:W]
                            else:
                                src = x4_r[0:K, g0:g0 + Gt, h0 + kh:h0 + kh + Ht, kw:kw + W]
                            s = scratch_pool.tile([K, 512], dt.bfloat16, name="s", tag="s")
                            s_v = s[:].rearrange("p (g a b) -> p g a b", g=Gt, a=Ht, b=W)
                            nc.vector.tensor_scalar(
                                out=s_v, in0=src,
                                scalar1=wcol_t[li][0:K, co * 9 + tap: co * 9 + tap + 1],
                                scalar2=0.0, op0=Alu.add, op1=Alu.max,
                            )
                            nc.tensor.matmul(
                                out=ps[:, :], lhsT=z_t[li][0:K, zoff:zoff + 128],
                                rhs=s[:, :], start=False, stop=(mm == n_mm - 1),
                            )
                            mm += 1

                    ps_r = ps[:].rearrange("p (g a b) -> p g a b", g=Gt, a=Ht, b=W)
                    if not l["pool"]:
                        # fused BN+ReLU straight into next layer's padded buffer
                        dst = x4n_r[:, g0:g0 + Gt, h0 + 1:h0 + 1 + Ht, 1:1 + W]
                        nc.scalar.activation(
                            out=dst, in_=ps_r, func=Act.Relu,
                            bias=bnb_t[li][:, 0:1], scale=bns_t[li][:, 0:1],
                        )
                        for gi in range(Gt):
                            nc.gpsimd.dma_start(
                                out=dupn_r[:, g0 + gi, h0 + 1:h0 + 1 + Ht, 0:W],
                                in_=x4n_r[:, g0 + gi, h0 + 1:h0 + 1 + Ht, 1:1 + W],
                            )
                    else:
                        Hh, Wh = Ht // 2, W // 2
                        ps5 = ps[:].rearrange("p (g a t b) -> p g a t b", g=Gt, a=Hh, t=2, b=W)
                        # psum holds +sum|x-w|; the activation is its negation, so
                        # maxpool on the activation = MIN on psum.
                        # Only one TT operand may live in PSUM: stage even rows
                        # through SBUF first.
                        t0 = tmp0_pool.tile([128, 256], dt.float32, name="t0", tag="t0")
                        t0_r = t0[:].rearrange("p (g a b) -> p g a b", g=Gt, a=Hh, b=W)
                        nc.vector.tensor_copy(out=t0_r, in_=ps5[:, :, :, 0, :])
                        t1 = tmp1_pool.tile([128, 256], dt.float32, name="t1", tag="t1")
                        t1_r = t1[:].rearrange("p (g a b) -> p g a b", g=Gt, a=Hh, b=W)
                        nc.vector.tensor_tensor(
                            out=t1_r, in0=t0_r, in1=ps5[:, :, :, 1, :], op=Alu.min)
                        t15 = t1[:].rearrange("p (g a b t) -> p g a b t", g=Gt, a=Hh, b=Wh, t=2)
                        t2 = tmp2_pool.tile([128, 128], dt.float32, name="t2", tag="t2")
                        t2_r = t2[:].rearrange("p (g a b) -> p g a b", g=Gt, a=Hh, b=Wh)
                        nc.vector.tensor_tensor(
                            out=t2_r, in0=t15[:, :, :, :, 0], in1=t15[:, :, :, :, 1], op=Alu.min)
                        if last:
                            # final: BN+ReLU to fp32 staging, then DMA out
                            os_r = out_sb[:].rearrange("p (g a b) -> p g a b", g=N_LOC, a=4, b=4)
                            nc.scalar.activation(
                                out=os_r[:, g0:g0 + Gt, :, :], in_=t2_r, func=Act.Relu,
                                bias=bnb_t[li][:, 0:1], scale=bns_t[li][:, 0:1],
                            )
                        else:
                            te = tmpe_pool.tile([128, 128], dt.bfloat16, name="te", tag="te")
                            te_r = te[:].rearrange("p (g a b) -> p g a b", g=Gt, a=Hh, b=Wh)
                            nc.scalar.activation(
                                out=te_r, in_=t2_r, func=Act.Relu,
                                bias=bnb_t[li][:, 0:1], scale=bns_t[li][:, 0:1],
                            )
                            # regroup images into next layer's (nsub', G') packing
                            nsub2, cin2 = ln["nsub"], ln["Cin"]
                            for gi in range(Gt):
                                for isub in range(nsub):
                                    i_img = (g0 + gi) * nsub + isub
                                    g2, is2 = i_img // nsub2, i_img % nsub2
                                    src_b = te_r[isub * Cout:(isub + 1) * Cout, gi, :, :]
                                    h1 = h0 // 2 + 1
                                    p2 = is2 * cin2
                                    dst_b = x4n_r[p2:p2 + Cout, g2, h1:h1 + Hh, 1:1 + Wh]
                                    nc.gpsimd.dma_start(out=dst_b, in_=src_b)
                                    ddst_b = dupn_r[p2:p2 + Cout, g2, h1:h1 + Hh, 0:Wh]
                                    nc.gpsimd.dma_start(out=ddst_b, in_=src_b)

            # ---- final output DMA: out_sb [co, (img, 4, 4)] -> out[img, co, 4, 4]
            os_r = out_sb[:].rearrange("p (g a b) -> p g a b", g=N_LOC, a=4, b=4)
            for i in range(N_LOC):
                nc.gpsimd.dma_start(out=out_h[i], in_=os_r[:, i, :, :])

    nc.finalize()
    return nc


def get_nc():
    if "nc" not in _NC_CACHE:
        _NC_CACHE["nc"] = _build_nc()
    return _NC_CACHE["nc"]


def kernel(x, params):
    global LAST_RESULTS
    from concourse.bass_utils import run_bass_kernel_spmd

    x = np.asarray(x, np.float32)
    tabs = _prep_tables(params)
    nc = get_nc()

    in_maps = []
    for c in range(N_CORES):
        m = {"x": np.ascontiguousarray(x[c * N_LOC:(c + 1) * N_LOC])}
        for li, t in enumerate(tabs):
            m[f"wcol{li}"] = t["wcol"]
            m[f"z{li}"] = t["Z"]
            m[f"zn{li}"] = t["zn"]
            m[f"bns{li}"] = t["bns"]
            m[f"bnb{li}"] = t["bnb"]
        in_maps.append(m)

    trace = bool(int(os.environ.get("KERNEL_TRACE", "0")))
    kw = {}
    if trace:
        kw["tmpdir"] = os.environ.get("KERNEL_TRACE_DIR") or None
    res = run_bass_kernel_spmd(nc, in_maps, list(range(N_CORES)), trace=trace, **kw)
    LAST_RESULTS = res
    out = np.concatenate([np.asarray(res.results[c]["out"], np.float32) for c in range(N_CORES)], axis=0)
    return out


# revision 17
# speedup vs baseline: 1.4930x; 1.4930x over previous
"""Trainium2 Bass kernel for nn_AdderConv (8-layer AdderNet CNN).

Problem: 8 adder-conv blocks (out = -sum |x-w|, 3x3, pad 1) + eval-mode BN +
ReLU, MaxPool2d(2,2) after blocks 1,3,7. Input x [64,3,32,32] f32, output
[64,128,4,4] f32. Pure data parallelism: 8 images per NeuronCore.

Algorithm (per core):
  - Activations live in SBUF as [(isub*Cin+ci) partitions, (g, Hp, Wp)] bf16
    with zero borders; nsub images packed on partitions so K = Cin*nsub <= 128
    and M = Cout*nsub == 128 for every layer.
  - sum_j |x_j - w_j|  =  2*sum_j relu(x_j - w_j) - sum_j x_j + sum_j w_j.
    VectorE tensor_scalar (op0=add with -w, op1=max 0) produces
    relu(x - w[co]) scratch tiles in the DVE 4x bf16 perf mode; TensorE
    reduces them across partitions with a one-hot selector matmul (value 2.0)
    accumulating into a PSUM bank [128, 512].  The -sum x term rides
    co-independent matmuls against a -1 selector; +sum w folds into the BN
    bias host-side.
  - ScalarE evicts PSUM with fused BN+ReLU (scale = -gamma/sqrt(var+eps)).
    MaxPool runs on PSUM before BN (valid: scale of the negated sum is
    positive... i.e. inv>0) as MIN over the raw sums, via two tensor_tensor
    steps.
  - K-packing: L0 uses a materialized im2col (all 27 taps -> K=108, one
    matmul per (co, qtile)); L2/L4 pack tap pairs (K=128, 5 chunks instead
    of 9); other layers use per-tap shifted views of the padded buffer
    directly (K=128).
  - A +1-element-shifted duplicate buffer keeps kw=1 taps 4-byte aligned so
    the DVE stays in its packed perf modes.

Weight/BN/selector tables are prepared host-side (cheap, O(weights)) and
shipped as extra DRAM inputs; all x-dependent compute runs on device.
"""

import os
import numpy as np

# ---------------- hardcoded problem config ----------------
N_CORES = 8
N_LOC = 8  # images per core
CONFIGS = [(3, 32), (32, 32), (32, 64), (64, 64), (64, 128), (128, 128), (128, 128), (128, 128)]
POOL_AFTER = (1, 3, 7)
BN_EPS = 1e-5

_LAYERS = []
for _i, (_cin, _cout) in enumerate(CONFIGS):
    _H = 32 if _i < 2 else (16 if _i < 4 else 8)
    _nsub = 4 if _i < 2 else (2 if _i < 4 else 1)
    _mode = "xcol27" if _i == 0 else ("pairs" if _i in (2, 4) else "native")
    _LAYERS.append(dict(
        Cin=_cin, Cout=_cout, H=_H, W=_H, nsub=_nsub, G=N_LOC // _nsub,
        Hp=_H + 2, Wp=_H + 2, SZ=(_H + 2) * (_H + 2),
        K=_cin * _nsub, M=_cout * _nsub, pool=(_i in POOL_AFTER), mode=_mode,
    ))
for _l in _LAYERS:
    assert _l["M"] == 128


def _qtiles(l):
    """q-tiles of 512 positions: list of (g0, Gt, h0, Ht)."""
    H, W, G = l["H"], l["W"], l["G"]
    if H == 32:
        return [(g, 1, h0, 16) for g in range(G) for h0 in (0, 16)]
    elif H == 16:
        return [(g0, 2, 0, 16) for g0 in (0, 2)]
    else:
        return [(0, 8, 0, 8)]


_PAIRS = [(0, 1), (2, 3), (4, 5), (6, 7)]  # tap pair chunks; tap 8 stays native


# ---------------- host-side table prep ----------------

def _prep_tables(params):
    import ml_dtypes
    bf16 = ml_dtypes.bfloat16
    tabs = []
    for li, l in enumerate(_LAYERS):
        p = params[li]
        w = np.asarray(p["w"], np.float32)          # [Cout, Cin, 3, 3]
        gamma = np.asarray(p["gamma"], np.float32)
        beta = np.asarray(p["beta"], np.float32)
        mean = np.asarray(p["mean"], np.float32)
        var = np.asarray(p["var"], np.float32)
        inv = gamma / np.sqrt(var + BN_EPS)
        assert np.all(inv > 0), "pool-before-BN requires positive BN scale"
        Cin, Cout, nsub, K = l["Cin"], l["Cout"], l["nsub"], l["K"]
        wf = w.reshape(Cout, Cin, 9)
        t = {}
        if l["mode"] == "xcol27":
            # rows r = tap*12 + isub*3 + ci
            Kx = 9 * K
            r = np.arange(Kx)
            tap, isub, ci = r // K, (r % K) // Cin, r % Cin
            t["wcol"] = (-wf[:, ci, tap].T).astype(np.float32)          # [Kx, Cout]
            F0 = Cout - 1
            Z = np.zeros((Kx, F0 + 128), np.float32)
            Z[r, F0 + isub * Cout] = 2.0
            t["Z"] = Z.astype(bf16)
            zn = np.zeros((Kx, 128), np.float32)
            zn[isub[:, None] == (np.arange(128) // Cout)[None, :]] = -1.0
            t["zn"] = zn.astype(bf16)
        else:
            # native rows k = isub*Cin + ci
            k = np.arange(K)
            isub, ci = k // Cin, k % Cin
            # wcol[k, co*9+tap]
            t["wcol"] = (-wf[:, ci, :].transpose(1, 0, 2).reshape(K, Cout * 9)).astype(np.float32)
            F0 = Cout - 1
            Z = np.zeros((K, F0 + 128), np.float32)
            Z[k, F0 + isub * Cout] = 2.0
            t["Z"] = Z.astype(bf16)
            zn = np.zeros((K, 128), np.float32)
            zn[isub[:, None] == (np.arange(128) // Cout)[None, :]] = -1.0
            t["zn"] = zn.astype(bf16)
            if l["mode"] == "pairs":
                # rows r: [0:K] tap 2c, [K:2K] tap 2c+1; wcolp[r, co*4+c]
                r = np.arange(2 * K)
                rh = r % K
                isub2, ci2 = rh // Cin, rh % Cin
                wcolp = np.zeros((2 * K, Cout * 4), np.float32)
                for c, (ta, tb) in enumerate(_PAIRS):
                    tap_of_r = np.where(r < K, ta, tb)
                    wcolp[:, c::4] = -wf[:, ci2, tap_of_r].T
                t["wcolp"] = wcolp
                Zp = np.zeros((2 * K, F0 + 128), np.float32)
                Zp[r, F0 + isub2 * Cout] = 2.0
                t["Zp"] = Zp.astype(bf16)
        co_of_m = np.arange(128) % Cout
        W1 = w.reshape(Cout, -1).sum(axis=1)
        t["bns"] = (-inv[co_of_m]).reshape(128, 1).astype(np.float32)
        t["bnb"] = (beta - inv * (mean + W1))[co_of_m].reshape(128, 1).astype(np.float32)
        tabs.append(t)
    return tabs


_TAB_SHAPES = {}  # name -> (shape, dtype_str) filled by _build_nc


# ---------------- bass program ----------------

_NC_CACHE = {}
LAST_RESULTS = None  # BassKernelResults of the most recent run (for profiling)


def _build_nc():
    import concourse.bacc as bacc
    import concourse.tile as tile
    import concourse.mybir as mybir
    dt = mybir.dt
    Alu = mybir.AluOpType
    Act = mybir.ActivationFunctionType

    nc = bacc.Bacc(None)

    x_h = nc.declare_dram_parameter("x", [N_LOC, 3, 32, 32], dt.float32, isOutput=False)
    out_h = nc.declare_dram_parameter("out", [N_LOC, 128, 4, 4], dt.float32, isOutput=True)

    tab_h = []  # per-layer dict name->handle
    for li, l in enumerate(_LAYERS):
        K, Cout = l["K"], l["Cout"]
        F0 = Cout - 1
        hs = {}
        if l["mode"] == "xcol27":
            Kx = 9 * K
            hs["wcol"] = ([Kx, Cout], dt.float32)
            hs["Z"] = ([Kx, F0 + 128], dt.bfloat16)
            hs["zn"] = ([Kx, 128], dt.bfloat16)
        else:
            hs["wcol"] = ([K, Cout * 9], dt.float32)
            hs["Z"] = ([K, F0 + 128], dt.bfloat16)
            hs["zn"] = ([K, 128], dt.bfloat16)
            if l["mode"] == "pairs":
                hs["wcolp"] = ([2 * K, Cout * 4], dt.float32)
                hs["Zp"] = ([2 * K, F0 + 128], dt.bfloat16)
        hs["bns"] = ([128, 1], dt.float32)
        hs["bnb"] = ([128, 1], dt.float32)
        handles = {}
        for nm, (shp, dty) in hs.items():
            handles[nm] = nc.declare_dram_parameter(f"{nm}{li}", shp, dty, isOutput=False)
        tab_h.append((hs, handles))

    with tile.TileContext(nc) as tc:
        with (
            tc.tile_pool(name="persist", bufs=1) as persist,
            tc.tile_pool(name="scratch", bufs=10) as scratch_pool,
            tc.tile_pool(name="psum", bufs=3, space="PSUM") as psum_pool,
            tc.tile_pool(name="tmp0", bufs=2) as tmp0_pool,
            tc.tile_pool(name="tmp1", bufs=2) as tmp1_pool,
            tc.tile_pool(name="tmp2", bufs=2) as tmp2_pool,
            tc.tile_pool(name="tmpe", bufs=2) as tmpe_pool,
            tc.tile_pool(name="xcolp", bufs=4) as xcolp_pool,
        ):
            # ---- persistent buffers ----
            x4, dup = [], []
            for li, l in enumerate(_LAYERS):
                shp = [l["K"], l["G"] * l["SZ"]]
                x4.append(persist.tile(shp, dt.bfloat16, name=f"x4_{li}", tag=f"x4_{li}"))
                dup.append(persist.tile(shp, dt.bfloat16, name=f"dup_{li}", tag=f"dup_{li}"))
            tab_t = []
            for li, (hs, handles) in enumerate(tab_h):
                tt = {}
                for nm, (shp, dty) in hs.items():
                    tt[nm] = persist.tile(shp, dty, name=f"{nm}t{li}", tag=f"{nm}t{li}")
                tab_t.append(tt)
            stage = persist.tile([12, 2 * 32 * 32], dt.float32, tag="stage")
            xcol0 = persist.tile([108, 2 * 32 * 32], dt.bfloat16, tag="xcol0")
            out_sb = persist.tile([128, N_LOC * 16], dt.float32, tag="out_sb")

            # ---- load tables ----
            for li, (hs, handles) in enumerate(tab_h):
                for nm in hs:
                    nc.gpsimd.dma_start(out=tab_t[li][nm][:], in_=handles[nm][:])

            # ---- zero-init padded buffers; dup initialised by copy ----
            for li in range(8):
                nc.gpsimd.memset(x4[li][:], 0.0)
            for li in range(1, 8):
                nc.gpsimd.dma_start(out=dup[li][:], in_=x4[li][:])

            # ---- input: DMA x into dense fp32 stage, convert+pad to bf16 ----
            # stage rows k = isub*3+ci, free (g=2, 32, 32); image i = g*4+isub
            stage_r = stage[:].rearrange("p (g a b) -> p g a b", g=2, a=32, b=32)
            x_ap = x_h[:]
            for k in range(12):
                isub, ci = k // 3, k % 3
                for g in range(2):
                    nc.gpsimd.dma_start(
                        out=stage_r[k:k + 1, g, :, :],
                        in_=x_ap[g * 4 + isub, ci:ci + 1, :, :],
                    )
            x40_r = x4[0][:].rearrange("p (g a b) -> p g a b", g=2, a=34, b=34)
            nc.vector.tensor_copy(out=x40_r[0:12, :, 1:33, 1:33], in_=stage_r)

            def tap_view(li_l, x4_r, dup_r, K, g0, Gt, h0, Ht, kh, kw, W):
                if kw == 1:
                    base, c0 = dup_r, 0
                else:
                    base, c0 = x4_r, kw
                if Gt == 1:
                    return base[0:K, g0, h0 + kh:h0 + kh + Ht, c0:c0 + W]
                return base[0:K, g0:g0 + Gt, h0 + kh:h0 + kh + Ht, c0:c0 + W]

            # ---- layers ----
            for li, l in enumerate(_LAYERS):
                K, Cout, nsub, G = l["K"], l["Cout"], l["nsub"], l["G"]
                H, W, Hp, Wp, SZ = l["H"], l["W"], l["Hp"], l["Wp"], l["SZ"]
                F0 = Cout - 1
                tt = tab_t[li]
                x4_r = x4[li][:].rearrange("p (g a b) -> p g a b", g=G, a=Hp, b=Wp)
                dup_r = dup[li][:].rearrange("p (g a b) -> p g a b", g=G, a=Hp, b=Wp)
                last = (li == 7)
                if not last:
                    ln = _LAYERS[li + 1]
                    x4n_r = x4[li + 1][:].rearrange("p (g a b) -> p g a b", g=ln["G"], a=ln["Hp"], b=ln["Wp"])
                    dupn_r = dup[li + 1][:].rearrange("p (g a b) -> p g a b", g=ln["G"], a=ln["Hp"], b=ln["Wp"])

                # ---- materialize K-packed im2col buffers ----
                if l["mode"] == "xcol27":
                    for tap in range(9):
                        kh, kw = tap // 3, tap % 3
                        for g in range(2):
                            nc.gpsimd.dma_start(
                                out=xcol0[:].rearrange("p (g a b) -> p g a b", g=2, a=32, b=32)[
                                    tap * 12:(tap + 1) * 12, g, :, :],
                                in_=x4_r[0:12, g, kh:kh + 32, kw:kw + 32],
                            )
                    qsrc = xcol0
                elif l["mode"] == "pairs":
                    Q_all = G * H * W
                    xcp = []
                    for c, (ta, tb) in enumerate(_PAIRS):
                        xc = xcolp_pool.tile([128, Q_all], dt.bfloat16, name=f"xc{li}_{c}", tag="xc")
                        xc_r = xc[:].rearrange("p (g a b) -> p g a b", g=G, a=H, b=W)
                        for half, tap in ((0, ta), (1, tb)):
                            kh, kw = tap // 3, tap % 3
                            for g in range(G):
                                nc.gpsimd.dma_start(
                                    out=xc_r[half * K:half * K + K, g, :, :],
                                    in_=x4_r[0:K, g, kh:kh + H, kw:kw + W],
                                )
                        xcp.append(xc)

                for (g0, Gt, h0, Ht) in _qtiles(l):
                    ps = psum_pool.tile([128, 512], dt.float32, name="ps", tag="ps")
                    FD = Gt * Ht * W

                    if l["mode"] == "xcol27":
                        # one zn matmul covers all 27 taps
                        qoff = (g0 * 1024) + h0 * 32
                        n_mm = 1 + Cout
                        nc.tensor.matmul(out=ps[:, :], lhsT=tt["zn"][0:108, 0:128],
                                         rhs=qsrc[0:108, qoff:qoff + 512],
                                         start=True, stop=False)
                        mm = 1
                        for co in range(Cout):
                            s = scratch_pool.tile([108, 512], dt.bfloat16, name="s", tag="s")
                            nc.vector.tensor_scalar(
                                out=s[:, :], in0=qsrc[0:108, qoff:qoff + 512],
                                scalar1=tt["wcol"][0:108, co:co + 1],
                                scalar2=0.0, op0=Alu.add, op1=Alu.max,
                            )
                            nc.tensor.matmul(
                                out=ps[:, :], lhsT=tt["Z"][0:108, F0 - co:F0 - co + 128],
                                rhs=s[:, :], start=False, stop=(mm == n_mm - 1),
                            )
                            mm += 1
                    else:
                        n_mm = 9 + Cout * (5 if l["mode"] == "pairs" else 9)
                        mm = 0
                        for tap in range(9):
                            kh, kw = tap // 3, tap % 3
                            src = tap_view(l, x4_r, dup_r, K, g0, Gt, h0, Ht, kh, kw, W)
                            nc.tensor.matmul(
                                out=ps[:, :], lhsT=tt["zn"][0:K, 0:128],
                                rhs=src, start=(mm == 0), stop=False,
                            )
                            mm += 1
                        if l["mode"] == "pairs":
                            qoff = g0 * H * W + h0 * W
                            for co in range(Cout):
                                for c in range(4):
                                    s = scratch_pool.tile([128, 512], dt.bfloat16, name="s", tag="s")
                                    nc.vector.tensor_scalar(
                                        out=s[:, :], in0=xcp[c][0:128, qoff:qoff + 512],
                                        scalar1=tt["wcolp"][0:128, co * 4 + c:co * 4 + c + 1],
                                        scalar2=0.0, op0=Alu.add, op1=Alu.max,
                                    )
                                    nc.tensor.matmul(
                                        out=ps[:, :], lhsT=tt["Zp"][0:128, F0 - co:F0 - co + 128],
                                        rhs=s[:, :], start=False, stop=False,
                                    )
                                    mm += 1
                                # tap 8 native
                                src = tap_view(l, x4_r, dup_r, K, g0, Gt, h0, Ht, 2, 2, W)
                                s = scratch_pool.tile([K, 512], dt.bfloat16, name="s", tag="s")
                                nc.vector.tensor_scalar(
                                    out=s[:, :], in0=src,
                                    scalar1=tt["wcol"][0:K, co * 9 + 8:co * 9 + 9],
                                    scalar2=0.0, op0=Alu.add, op1=Alu.max,
                                )
                                nc.tensor.matmul(
                                    out=ps[:, :], lhsT=tt["Z"][0:K, F0 - co:F0 - co + 128],
                                    rhs=s[:, :], start=False, stop=(mm == n_mm - 1),
                                )
                                mm += 1
                        else:
                            for co in range(Cout):
                                for tap in range(9):
                                    kh, kw = tap // 3, tap % 3
                                    src = tap_view(l, x4_r, dup_r, K, g0, Gt, h0, Ht, kh, kw, W)
                                    s = scratch_pool.tile([K, 512], dt.bfloat16, name="s", tag="s")
                                    nc.vector.tensor_scalar(
                                        out=s[:, :], in0=src,
                                        scalar1=tt["wcol"][0:K, co * 9 + tap:co * 9 + tap + 1],
                                        scalar2=0.0, op0=Alu.add, op1=Alu.max,
                                    )
                                    nc.tensor.matmul(
                                        out=ps[:, :], lhsT=tt["Z"][0:K, F0 - co:F0 - co + 128],
                                        rhs=s[:, :], start=False, stop=(mm == n_mm - 1),
                                    )
                                    mm += 1

                    # ---- eviction ----
                    if not l["pool"]:
                        ps_r = ps[:].rearrange("p (g a b) -> p g a b", g=Gt, a=Ht, b=W)
                        if Gt == 1:
                            dst = x4n_r[:, g0, h0 + 1:h0 + 1 + Ht, 1:1 + W]
                            psv = ps[:].rearrange("p (a b) -> p a b", a=Ht, b=W)
                        else:
                            dst = x4n_r[:, g0:g0 + Gt, h0 + 1:h0 + 1 + Ht, 1:1 + W]
                            psv = ps_r
                        nc.scalar.activation(
                            out=dst, in_=psv, func=Act.Relu,
                            bias=tt["bnb"][:, 0:1], scale=tt["bns"][:, 0:1],
                        )
                        for gi in range(Gt):
                            nc.gpsimd.dma_start(
                                out=dupn_r[:, g0 + gi, h0 + 1:h0 + 1 + Ht, 0:W],
                                in_=x4n_r[:, g0 + gi, h0 + 1:h0 + 1 + Ht, 1:1 + W],
                            )
                    else:
                        Hh, Wh = Ht // 2, W // 2
                        ps5 = ps[:].rearrange("p (g a t b) -> p g a t b", g=Gt, a=Hh, t=2, b=W)
                        # psum holds +sum|x-w|; activation is its negation, so
                        # maxpool = MIN on psum.  Only one TT operand may be in
                        # PSUM: stage even rows through SBUF first.
                        t0 = tmp0_pool.tile([128, 256], dt.float32, name="t0", tag="t0")
                        t0_r = t0[:].rearrange("p (g a b) -> p g a b", g=Gt, a=Hh, b=W)
                        nc.vector.tensor_copy(out=t0_r, in_=ps5[:, :, :, 0, :])
                        t1 = tmp1_pool.tile([128, 256], dt.float32, name="t1", tag="t1")
                        t1_r = t1[:].rearrange("p (g a b) -> p g a b", g=Gt, a=Hh, b=W)
                        nc.vector.tensor_tensor(
                            out=t1_r, in0=t0_r, in1=ps5[:, :, :, 1, :], op=Alu.min)
                        t15 = t1[:].rearrange("p (g a b t) -> p g a b t", g=Gt, a=Hh, b=Wh, t=2)
                        t2 = tmp2_pool.tile([128, 128], dt.float32, name="t2", tag="t2")
                        t2_r = t2[:].rearrange("p (g a b) -> p g a b", g=Gt, a=Hh, b=Wh)
                        nc.vector.tensor_tensor(
                            out=t2_r, in0=t15[:, :, :, :, 0], in1=t15[:, :, :, :, 1], op=Alu.min)
                        if last:
                            os_r = out_sb[:].rearrange("p (g a b) -> p g a b", g=N_LOC, a=4, b=4)
                            nc.scalar.activation(
                                out=os_r[:, g0:g0 + Gt, :, :], in_=t2_r, func=Act.Relu,
                                bias=tt["bnb"][:, 0:1], scale=tt["bns"][:, 0:1],
                            )
                        else:
                            te = tmpe_pool.tile([128, 128], dt.bfloat16, name="te", tag="te")
                            te_r = te[:].rearrange("p (g a b) -> p g a b", g=Gt, a=Hh, b=Wh)
                            nc.scalar.activation(
                                out=te_r, in_=t2_r, func=Act.Relu,
                                bias=tt["bnb"][:, 0:1], scale=tt["bns"][:, 0:1],
                            )
                            nsub2, cin2 = ln["nsub"], ln["Cin"]
                            for gi in range(Gt):
                                for isub in range(nsub):
                                    i_img = (g0 + gi) * nsub + isub
                                    g2, is2 = i_img // nsub2, i_img % nsub2
                                    src_b = te_r[isub * Cout:(isub + 1) * Cout, gi, :, :]
                                    h1 = h0 // 2 + 1
                                    p2 = is2 * cin2
                                    dst_b = x4n_r[p2:p2 + Cout, g2, h1:h1 + Hh, 1:1 + Wh]
                                    nc.gpsimd.dma_start(out=dst_b, in_=src_b)
                                    ddst_b = dupn_r[p2:p2 + Cout, g2, h1:h1 + Hh, 0:Wh]
                                    nc.gpsimd.dma_start(out=ddst_b, in_=src_b)

            # ---- final output DMA: out_sb [co, (img, 4, 4)] -> out[img, co, 4, 4]
            os_r = out_sb[:].rearrange("p (g a b) -> p g a b", g=N_LOC, a=4, b=4)
            for i in range(N_LOC):
                nc.gpsimd.dma_start(out=out_h[i], in_=os_r[:, i, :, :])

    nc.finalize()
    return nc


def get_nc():
    if "nc" not in _NC_CACHE:
        _NC_CACHE["nc"] = _build_nc()
    return _NC_CACHE["nc"]


def kernel(x, params):
    global LAST_RESULTS
    from concourse.bass_utils import run_bass_kernel_spmd

    x = np.asarray(x, np.float32)
    tabs = _prep_tables(params)
    nc = get_nc()

    in_maps = []
    for c in range(N_CORES):
        m = {"x": np.ascontiguousarray(x[c * N_LOC:(c + 1) * N_LOC])}
        for li, t in enumerate(tabs):
            for nm, arr in t.items():
                m[f"{nm}{li}"] = arr
        in_maps.append(m)

    trace = bool(int(os.environ.get("KERNEL_TRACE", "0")))
    kw = {}
    if trace:
        kw["tmpdir"] = os.environ.get("KERNEL_TRACE_DIR") or None
    res = run_bass_kernel_spmd(nc, in_maps, list(range(N_CORES)), trace=trace, **kw)
    LAST_RESULTS = res
    out = np.concatenate([np.asarray(res.results[c]["out"], np.float32) for c in range(N_CORES)], axis=0)
    return out
